# revision 4
# baseline (speedup 1.0000x reference)
"""AdaFace loss on 8 TRN2 NeuronCores, class-parallel.

Strategy: shard the 100k weight rows (classes) across 8 cores. Host
pre-normalizes rows, transposes to [D, C_shard] and casts to bf16 so each
core streams its 6.55MB shard straight into TensorE as the matmul moving
operand. Since |logit| = |32*cos| <= 32, a fixed shift of 32 replaces the
per-row max of a standard log-softmax, so the only collective is a single
2KB AllReduce of per-sample partial sum-exp. The target-class margin term
(cos(theta+m), needs only sqrt, no arccos) is folded into a per-sample
correction vector computed on host in f64.

Device per core: 200 bf16 matmuls -> PSUM [128b, 512c] cosine tiles,
ScalarE exp(32x-32) with accum_out row-sums, AllReduce(512 f32), then
ln + weighted-dot epilogue -> scalar.
"""

import numpy as np
import ml_dtypes

import concourse.bass as bass
import concourse.tile as tile
from concourse import bacc, mybir
from concourse.bass_utils import run_bass_kernel_spmd
from concourse.masks import make_identity

B = 512
D = 256
C = 100000
NCORES = 8
CSH = C // NCORES          # 12500 classes per core
F = 512                    # classes per matmul (one PSUM bank)
NT = 25                    # class tiles per core -> CPAD = NT * F
CPAD = NT * F              # padded classes per core
NPAD_TOT = (CPAD - CSH) * NCORES

M0 = 0.5
M_MIN = 0.25
SCALE = 32.0
SHIFT = 32.0               # fixed log-softmax shift (|logits| <= SCALE)

f32 = mybir.dt.float32
bf16 = mybir.dt.bfloat16

_cached_nc = None
_last_results = None


def _build():
    global _cached_nc
    if _cached_nc is not None:
        return _cached_nc

    nc = bacc.Bacc(
        "TRN2", target_bir_lowering=False, debug=False, num_devices=NCORES
    )

    wnT_d = nc.dram_tensor("wnT", [D, CPAD], bf16, kind="ExternalInput")
    featnT_d = nc.dram_tensor("featnT", [D, B], bf16, kind="ExternalInput")
    corr_d = nc.dram_tensor("corr", [1, B], f32, kind="ExternalInput")
    coef_d = nc.dram_tensor("coef", [1, B], f32, kind="ExternalInput")
    out_d = nc.dram_tensor("out", [1, 1], f32, kind="ExternalOutput")

    NBC = B // 128  # 4 batch chunks

    with tile.TileContext(nc) as tc:
        with (
            tc.tile_pool(name="fpool", bufs=1) as fpool,
            tc.tile_pool(name="wpool", bufs=4) as wpool,
            tc.tile_pool(name="epool", bufs=3) as epool,
            tc.tile_pool(name="misc", bufs=1) as misc,
            tc.tile_pool(name="psum", bufs=7, space="PSUM") as psum,
            tc.tile_pool(name="psumt", bufs=1, space="PSUM") as psumt,
            tc.tile_pool(name="dram", bufs=1, space="DRAM") as dram,
        ):
            # features^T (normalized, bf16): two K-chunks of [128, 512]
            fT = []
            for k in range(2):
                t = fpool.tile([128, B], bf16, tag=f"fT{k}")
                nc.sync.dma_start(out=t[:], in_=featnT_d[k * 128:(k + 1) * 128, :])
                fT.append(t)

            corr_s = misc.tile([1, B], f32)
            nc.sync.dma_start(out=corr_s[:], in_=corr_d[:])
            coef_s = misc.tile([1, B], f32)
            nc.sync.dma_start(out=coef_s[:], in_=coef_d[:])

            ident = misc.tile([128, 128], f32)
            make_identity(nc, ident[:])

            bias_m32 = misc.tile([128, 1], f32)
            nc.vector.memset(bias_m32[:], -SHIFT)
            zbias = misc.tile([1, 1], f32)
            nc.vector.memset(zbias[:], 0.0)

            # per-batch-chunk partial sums, one column per class tile
            SCW = 32  # column stride per batch chunk (>= NT)
            scol = misc.tile([128, NBC * SCW], f32)

            for t in range(NT):
                w0 = wpool.tile([128, F], bf16, tag="w0")
                w1 = wpool.tile([128, F], bf16, tag="w1")
                nc.sync.dma_start(out=w0[:], in_=wnT_d[0:128, t * F:(t + 1) * F])
                nc.sync.dma_start(out=w1[:], in_=wnT_d[128:256, t * F:(t + 1) * F])
                for bc in range(NBC):
                    ps = psum.tile([128, F], f32)
                    nc.tensor.matmul(
                        ps[:], fT[0][:, bc * 128:(bc + 1) * 128], w0[:],
                        start=True, stop=False,
                    )
                    nc.tensor.matmul(
                        ps[:], fT[1][:, bc * 128:(bc + 1) * 128], w1[:],
                        start=False, stop=True,
                    )
                    esc = epool.tile([128, F], bf16)
                    nc.scalar.activation(
                        esc[:], ps[:], mybir.ActivationFunctionType.Exp,
                        bias=bias_m32[:], scale=SCALE,
                        accum_out=scol[:, bc * SCW + t:bc * SCW + t + 1],
                    )

            # reduce the NT partial columns -> S_all [128, 4]
            S_all = misc.tile([128, NBC], f32)
            for bc in range(NBC):
                nc.vector.tensor_reduce(
                    S_all[:, bc:bc + 1],
                    scol[:, bc * SCW:bc * SCW + NT],
                    axis=mybir.AxisListType.X,
                    op=mybir.AluOpType.add,
                )

            # transpose [128, 4] -> [4, 128] so DRAM bounce is batch-ordered
            pt = psumt.tile([128, 128], f32)
            nc.tensor.transpose(pt[:NBC, :128], S_all[:], ident[:])

            S_t = misc.tile([NBC, 128], f32)
            nc.vector.tensor_copy(S_t[:], pt[:NBC, :128])

            bin_ = dram.tile([NBC, 128], f32)
            bout = dram.tile([NBC, 128], f32)
            nc.sync.dma_start(out=bin_[:], in_=S_t[:])
            nc.gpsimd.collective_compute(
                "AllReduce",
                mybir.AluOpType.add,
                replica_groups=[list(range(NCORES))],
                ins=[bin_.opt()],
                outs=[bout.opt()],
            )

            # epilogue: Z = S + corr; out = sum(coef * ln(Z))
            Zrow = misc.tile([1, B], f32)
            nc.sync.dma_start(
                out=Zrow[:], in_=bout[:].rearrange("a b -> () (a b)")
            )
            nc.vector.tensor_add(Zrow[:], Zrow[:], corr_s[:])
            logZ = misc.tile([1, B], f32)
            nc.scalar.activation(
                logZ[:], Zrow[:], mybir.ActivationFunctionType.Ln, bias=zbias[:]
            )
            prod = misc.tile([1, B], f32)
            res = misc.tile([1, 1], f32)
            nc.vector.tensor_mul(prod[:], logZ[:], coef_s[:])
            nc.vector.tensor_reduce(
                res[:], prod[:], axis=mybir.AxisListType.X,
                op=mybir.AluOpType.add,
            )
            nc.sync.dma_start(out=out_d[:], in_=res[:])

    nc.compile()
    _cached_nc = nc
    return nc


def kernel(features, weight, weights, labels):
    global _last_results
    features = np.asarray(features, dtype=np.float32)
    weight = np.asarray(weight, dtype=np.float32)
    weights = np.asarray(weights, dtype=np.float32)
    labels = np.asarray(labels).astype(np.int64)

    # ---- host-side per-sample terms (f64) ----
    f = features.astype(np.float64)
    norms = np.sqrt((f * f).sum(axis=1))
    lo, hi = norms.min(), norms.max()
    denom = max(hi - lo, 1e-8)
    margins = np.clip(M_MIN + (M0 - M_MIN) * (norms - lo) / denom, M_MIN, M0)
    feat_n = f / np.maximum(norms, 1e-12)[:, None]

    wlab = weight[labels].astype(np.float64)
    wlab_n = wlab / np.maximum(
        np.sqrt((wlab * wlab).sum(axis=1)), 1e-12
    )[:, None]
    cos_t = np.clip((feat_n * wlab_n).sum(axis=1), -1.0 + 1e-7, 1.0 - 1e-7)
    cos_m = cos_t * np.cos(margins) - np.sqrt(1.0 - cos_t * cos_t) * np.sin(
        margins
    )
    t_logit = SCALE * cos_m

    # correction: replace raw target term with margin term; cancel the
    # zero-padded classes' exp(0 - SHIFT) contributions exactly.
    corr = (
        np.exp(SCALE * cos_m - SHIFT)
        - np.exp(SCALE * cos_t - SHIFT)
        - NPAD_TOT * np.exp(-SHIFT)
    )
    coef = weights.astype(np.float64) / B
    B0 = float((coef * (SHIFT - t_logit)).sum())

    # ---- shard weight: normalize rows, transpose, cast bf16 ----
    wn = weight / np.maximum(
        np.linalg.norm(weight, axis=1, keepdims=True), 1e-12
    )
    featnT = np.ascontiguousarray(
        feat_n.T.astype(np.float32)
    ).astype(ml_dtypes.bfloat16)
    corr_f32 = corr.astype(np.float32).reshape(1, B)
    coef_f32 = coef.astype(np.float32).reshape(1, B)

    in_maps = []
    for i in range(NCORES):
        sh = wn[i * CSH:(i + 1) * CSH]  # [CSH, D]
        wt = np.zeros((D, CPAD), dtype=ml_dtypes.bfloat16)
        wt[:, :CSH] = sh.T.astype(ml_dtypes.bfloat16)
        in_maps.append(
            {
                "wnT": wt,
                "featnT": featnT,
                "corr": corr_f32,
                "coef": coef_f32,
            }
        )

    nc = _build()
    res = run_bass_kernel_spmd(nc, in_maps, list(range(NCORES)))
    _last_results = res
    out0 = float(np.asarray(res.results[0]["out"]).reshape(-1)[0])
    return np.array(out0 + B0, dtype=np.float32)


# revision 5
# speedup vs baseline: 1.9039x; 1.9039x over previous
"""AdaFace loss on 8 TRN2 NeuronCores, class-parallel.

Strategy: shard the 100k weight rows (classes) across 8 cores. Host
pre-normalizes rows, transposes to [D, C_shard] and casts to bf16 so each
core streams its shard straight into TensorE as the matmul moving
operand. Since |logit| = |32*cos| <= 32, a fixed shift of 32 replaces the
per-row max of a standard log-softmax, so no max collective is needed.
Each core returns per-(batch, class-tile) partial sums of exp(32c-32);
the host does the final O(B) combine: sum across cores/tiles, add the
margin-target correction (cos(theta+m) needs only sqrt, no arccos),
ln, weighted dot. No device collective at all.

Device per core: bf16 matmuls -> PSUM [128b, 1024c] cosine tiles (2
banks), ScalarE exp(32x-32) (f32 PSUM -> bf16 SBUF), VectorE 4x-mode
row-sum -> partial-sum columns, one DMA out.
"""

import numpy as np
import ml_dtypes

import concourse.bass as bass
import concourse.tile as tile
from concourse import bacc, mybir
from concourse.bass_utils import run_bass_kernel_spmd

B = 512
D = 256
C = 100000
NCORES = 8
CSH = C // NCORES          # 12500 classes per core
# class tiles per core: twelve 1024-wide + one 256-wide = 12544
TILES = [(i * 1024, 1024) for i in range(12)] + [(12288, 256)]
CPAD = 12544
NPAD_TOT = (CPAD - CSH) * NCORES

M0 = 0.5
M_MIN = 0.25
SCALE = 32.0
SHIFT = 32.0               # fixed log-softmax shift (|logits| <= SCALE)

f32 = mybir.dt.float32
bf16 = mybir.dt.bfloat16

NBC = B // 128             # 4 batch chunks
SCW = 16                   # scol column stride per batch chunk (>= len(TILES))

_cached_nc = None
_last_results = None


def _build():
    global _cached_nc
    if _cached_nc is not None:
        return _cached_nc

    nc = bacc.Bacc(
        "TRN2", target_bir_lowering=False, debug=False, num_devices=NCORES
    )

    wnT_d = nc.dram_tensor("wnT", [D, CPAD], bf16, kind="ExternalInput")
    featnT_d = nc.dram_tensor("featnT", [D, B], bf16, kind="ExternalInput")
    out_d = nc.dram_tensor("out", [128, NBC * SCW], f32, kind="ExternalOutput")

    with tile.TileContext(nc) as tc:
        with (
            tc.tile_pool(name="fpool", bufs=1) as fpool,
            tc.tile_pool(name="wpool", bufs=3) as wpool,
            tc.tile_pool(name="epool", bufs=3) as epool,
            tc.tile_pool(name="misc", bufs=1) as misc,
            tc.tile_pool(name="psum", bufs=4, space="PSUM") as psum,
        ):
            fT = []
            for k in range(2):
                t = fpool.tile([128, B], bf16, tag=f"fT{k}")
                nc.sync.dma_start(out=t[:], in_=featnT_d[k * 128:(k + 1) * 128, :])
                fT.append(t)

            bias_s = misc.tile([128, 1], f32)
            nc.vector.memset(bias_s[:], -SHIFT)

            scol = misc.tile([128, NBC * SCW], f32)

            for ti, (c0, cw) in enumerate(TILES):
                w0 = wpool.tile([128, 1024], bf16, tag="w0")
                w1 = wpool.tile([128, 1024], bf16, tag="w1")
                nc.sync.dma_start(out=w0[:, :cw], in_=wnT_d[0:128, c0:c0 + cw])
                nc.sync.dma_start(out=w1[:, :cw], in_=wnT_d[128:256, c0:c0 + cw])
                for bc in range(NBC):
                    ps = psum.tile([128, 1024], f32, tag="ps")
                    for j in range(0, cw, 512):
                        jw = min(512, cw - j)
                        nc.tensor.matmul(
                            ps[:, j:j + jw],
                            fT[0][:, bc * 128:(bc + 1) * 128],
                            w0[:, j:j + jw],
                            start=True, stop=False,
                        )
                        nc.tensor.matmul(
                            ps[:, j:j + jw],
                            fT[1][:, bc * 128:(bc + 1) * 128],
                            w1[:, j:j + jw],
                            start=False, stop=True,
                        )
                    esc = epool.tile([128, 1024], bf16, tag="esc")
                    nc.scalar.activation(
                        esc[:, :cw], ps[:, :cw],
                        mybir.ActivationFunctionType.Exp,
                        bias=bias_s[:], scale=SCALE,
                    )
                    nc.vector.tensor_reduce(
                        scol[:, bc * SCW + ti:bc * SCW + ti + 1],
                        esc[:, :cw],
                        axis=mybir.AxisListType.X,
                        op=mybir.AluOpType.add,
                    )

            nc.sync.dma_start(out=out_d[:], in_=scol[:])

    nc.compile()
    _cached_nc = nc
    return nc


def kernel(features, weight, weights, labels):
    global _last_results
    features = np.asarray(features, dtype=np.float32)
    weight = np.asarray(weight, dtype=np.float32)
    weights = np.asarray(weights, dtype=np.float32)
    labels = np.asarray(labels).astype(np.int64)

    # ---- host-side per-sample terms (f64) ----
    f = features.astype(np.float64)
    norms = np.sqrt((f * f).sum(axis=1))
    lo, hi = norms.min(), norms.max()
    denom = max(hi - lo, 1e-8)
    margins = np.clip(M_MIN + (M0 - M_MIN) * (norms - lo) / denom, M_MIN, M0)
    feat_n = f / np.maximum(norms, 1e-12)[:, None]

    wlab = weight[labels].astype(np.float64)
    wlab_n = wlab / np.maximum(
        np.sqrt((wlab * wlab).sum(axis=1)), 1e-12
    )[:, None]
    cos_t = np.clip((feat_n * wlab_n).sum(axis=1), -1.0 + 1e-7, 1.0 - 1e-7)
    cos_m = cos_t * np.cos(margins) - np.sqrt(1.0 - cos_t * cos_t) * np.sin(
        margins
    )
    t_logit = SCALE * cos_m

    # replace raw target term with margin term; cancel the zero-padded
    # classes' exp(0 - SHIFT) contributions.
    corr = (
        np.exp(SCALE * cos_m - SHIFT)
        - np.exp(SCALE * cos_t - SHIFT)
        - NPAD_TOT * np.exp(-SHIFT)
    )
    coef = weights.astype(np.float64) / B

    # ---- shard weight: normalize rows, transpose, cast bf16 ----
    wn = weight / np.maximum(
        np.linalg.norm(weight, axis=1, keepdims=True), 1e-12
    )
    featnT = np.ascontiguousarray(
        feat_n.T.astype(np.float32)
    ).astype(ml_dtypes.bfloat16)

    in_maps = []
    for i in range(NCORES):
        sh = wn[i * CSH:(i + 1) * CSH]  # [CSH, D]
        wt = np.zeros((D, CPAD), dtype=ml_dtypes.bfloat16)
        wt[:, :CSH] = sh.T.astype(ml_dtypes.bfloat16)
        in_maps.append({"wnT": wt, "featnT": featnT})

    nc = _build()
    res = run_bass_kernel_spmd(nc, in_maps, list(range(NCORES)))
    _last_results = res

    # ---- host combine: S[b] = sum over cores/tiles of partials ----
    S = np.zeros(B, dtype=np.float64)
    nt = len(TILES)
    for i in range(NCORES):
        sc = np.asarray(res.results[i]["out"], dtype=np.float64)  # [128, 64]
        for bc in range(NBC):
            S[bc * 128:(bc + 1) * 128] += sc[:, bc * SCW:bc * SCW + nt].sum(
                axis=1
            )

    Z = S + corr
    per = SHIFT + np.log(Z) - t_logit
    loss = float((coef * per).sum())
    return np.array(loss, dtype=np.float32)


# revision 6
# speedup vs baseline: 1.9225x; 1.0098x over previous
"""AdaFace loss on 8 TRN2 NeuronCores, class-parallel.

Strategy: shard the 100k weight rows (classes) across 8 cores. Host
pre-normalizes rows, transposes to [D, C_shard], scales by 8 and casts to
fp8e4 (scale keeps values out of the e4m3 subnormal range; the ScalarE
exp absorbs it: exp(0.5*x - 32) of the 64*cos matmul result). Since
|logit| <= 32, a fixed shift of 32 replaces the per-row max of a
standard log-softmax, so no max collective is needed. Each core returns
per-batch-chunk partial sums of exp(32c-32); the host does the final
O(B) combine: sum across cores, margin-target correction (cos(theta+m)
needs only sqrt, no arccos), ln, weighted dot. No device collective.

Device per core: the whole fp8 weight shard stays resident in SBUF
(24.5KB/partition); DoubleRow matmuls (K=256 per instruction) fill
4-bank PSUM tiles [128b, 2048c]; ScalarE exp -> bf16; VectorE 2x-mode
running adds + one final reduce per batch chunk; single DMA out.
"""

import numpy as np
import ml_dtypes

import concourse.bass as bass
import concourse.tile as tile
from concourse import bacc, mybir
from concourse.bass_utils import run_bass_kernel_spmd

B = 512
D = 256
C = 100000
NCORES = 8
CSH = C // NCORES          # 12500 classes per core
# class tiles per core: six 2048-wide + one 256-wide = 12544
TILES = [(i * 2048, 2048) for i in range(6)] + [(12288, 256)]
CPAD = 12544
NPAD_TOT = (CPAD - CSH) * NCORES

M0 = 0.5
M_MIN = 0.25
SCALE = 32.0
SHIFT = 32.0               # fixed log-softmax shift (|logits| <= SCALE)
FP8_PRESCALE = 8.0         # both operands scaled by 8 -> matmul gives 64*cos

f32 = mybir.dt.float32
bf16 = mybir.dt.bfloat16
fp8 = mybir.dt.float8e4

NBC = B // 128             # 4 batch chunks

_cached_nc = None
_last_results = None


def _build():
    global _cached_nc
    if _cached_nc is not None:
        return _cached_nc

    nc = bacc.Bacc(
        "TRN2", target_bir_lowering=False, debug=False, num_devices=NCORES
    )

    # [p, j, c] with contraction index k = j*128 + p
    wnT_d = nc.dram_tensor("wnT", [128, 2, CPAD], fp8, kind="ExternalInput")
    featnT_d = nc.dram_tensor("featnT", [128, 2, B], fp8, kind="ExternalInput")
    out_d = nc.dram_tensor("out", [128, NBC], f32, kind="ExternalOutput")

    with tile.TileContext(nc) as tc:
        with (
            tc.tile_pool(name="persist", bufs=1) as persist,
            tc.tile_pool(name="epool", bufs=3) as epool,
            tc.tile_pool(name="psum", bufs=2, space="PSUM") as psum,
        ):
            fsb = persist.tile([128, 2, B], fp8)
            nc.sync.dma_start(out=fsb[:], in_=featnT_d[:])

            wsb = persist.tile([128, 2, CPAD], fp8)
            # chunked loads so compute can start early
            NW = 8
            step = CPAD // NW  # 1568 = CPAD/8
            for ci in range(NW):
                nc.sync.dma_start(
                    out=wsb[:, :, ci * step:(ci + 1) * step],
                    in_=wnT_d[:, :, ci * step:(ci + 1) * step],
                )

            bias_s = persist.tile([128, 1], f32)
            nc.vector.memset(bias_s[:], -SHIFT)

            eacc = []
            for bc in range(NBC):
                t = persist.tile([128, 2048], bf16, tag=f"eacc{bc}")
                nc.vector.memset(t[:], 0.0)
                eacc.append(t)

            S_all = persist.tile([128, NBC], f32)

            for bc in range(NBC):
                lhs = fsb[:, :, bc * 128:(bc + 1) * 128]
                for c0, cw in TILES:
                    ps = psum.tile([128, 2048], f32, tag="ps")
                    for j in range(0, cw, 512):
                        jw = min(512, cw - j)
                        nc.tensor.matmul(
                            ps[:, j:j + jw],
                            lhs,
                            wsb[:, :, c0 + j:c0 + j + jw],
                            start=True, stop=True,
                            perf_mode=mybir.MatmulPerfMode.DoubleRow,
                        )
                    esc = epool.tile([128, 2048], bf16, tag="esc")
                    nc.scalar.activation(
                        esc[:, :cw], ps[:, :cw],
                        mybir.ActivationFunctionType.Exp,
                        bias=bias_s[:], scale=SCALE / (FP8_PRESCALE**2),
                    )
                    nc.vector.tensor_add(
                        eacc[bc][:, :cw], eacc[bc][:, :cw], esc[:, :cw]
                    )
                nc.vector.tensor_reduce(
                    S_all[:, bc:bc + 1],
                    eacc[bc][:],
                    axis=mybir.AxisListType.X,
                    op=mybir.AluOpType.add,
                )

            nc.sync.dma_start(out=out_d[:], in_=S_all[:])

    nc.compile()
    _cached_nc = nc
    return nc


def _host_prep(features, weight, weights, labels):
    """Everything O(B*D) / O(C*D) that is not the big matmul."""
    f = features.astype(np.float64)
    norms = np.sqrt((f * f).sum(axis=1))
    lo, hi = norms.min(), norms.max()
    denom = max(hi - lo, 1e-8)
    margins = np.clip(M_MIN + (M0 - M_MIN) * (norms - lo) / denom, M_MIN, M0)
    feat_n = f / np.maximum(norms, 1e-12)[:, None]

    wlab = weight[labels].astype(np.float64)
    wlab_n = wlab / np.maximum(
        np.sqrt((wlab * wlab).sum(axis=1)), 1e-12
    )[:, None]
    cos_t = np.clip((feat_n * wlab_n).sum(axis=1), -1.0 + 1e-7, 1.0 - 1e-7)
    cos_m = cos_t * np.cos(margins) - np.sqrt(1.0 - cos_t * cos_t) * np.sin(
        margins
    )
    t_logit = SCALE * cos_m
    corr = (
        np.exp(SCALE * cos_m - SHIFT)
        - np.exp(SCALE * cos_t - SHIFT)
        - NPAD_TOT * np.exp(-SHIFT)
    )
    coef = weights.astype(np.float64) / B
    return feat_n, corr, coef, t_logit


def _to_dr_layout(mat_t, width):
    """[D, X] f32 -> [128, 2, X] fp8 with k = j*128 + p."""
    a = mat_t.reshape(2, 128, width)          # [j, p, X]
    a = np.ascontiguousarray(a.transpose(1, 0, 2))  # [p, j, X]
    return a.astype(ml_dtypes.float8_e4m3)


def kernel(features, weight, weights, labels):
    global _last_results
    features = np.asarray(features, dtype=np.float32)
    weight = np.asarray(weight, dtype=np.float32)
    weights = np.asarray(weights, dtype=np.float32)
    labels = np.asarray(labels).astype(np.int64)

    feat_n, corr, coef, t_logit = _host_prep(features, weight, weights, labels)

    wn = weight / np.maximum(
        np.linalg.norm(weight, axis=1, keepdims=True), 1e-12
    )
    featnT = np.ascontiguousarray(feat_n.T.astype(np.float32)) * FP8_PRESCALE
    featnT8 = _to_dr_layout(featnT, B)

    in_maps = []
    for i in range(NCORES):
        sh = wn[i * CSH:(i + 1) * CSH]  # [CSH, D]
        wt = np.zeros((D, CPAD), dtype=np.float32)
        wt[:, :CSH] = sh.T * FP8_PRESCALE
        in_maps.append(
            {"wnT": _to_dr_layout(wt, CPAD), "featnT": featnT8}
        )

    nc = _build()
    res = run_bass_kernel_spmd(nc, in_maps, list(range(NCORES)))
    _last_results = res

    # ---- host combine ----
    S = np.zeros(B, dtype=np.float64)
    for i in range(NCORES):
        sc = np.asarray(res.results[i]["out"], dtype=np.float64)  # [128, 4]
        for bc in range(NBC):
            S[bc * 128:(bc + 1) * 128] += sc[:, bc]

    Z = S + corr
    per = SHIFT + np.log(Z) - t_logit
    loss = float((coef * per).sum())
    return np.array(loss, dtype=np.float32)


# revision 9
# speedup vs baseline: 1.9401x; 1.0092x over previous
"""AdaFace loss on 8 TRN2 NeuronCores, class-parallel.

Strategy: shard the 100k weight rows (classes) across 8 cores. Host
pre-normalizes rows, transposes to [D, C_shard], scales by 8 and casts to
fp8e4 (scale keeps values out of the e4m3 subnormal range; the ScalarE
exp absorbs it: exp(0.5*x - 32) of the 64*cos matmul result). Since
|logit| <= 32, a fixed shift of 32 replaces the per-row max of a
standard log-softmax, so no max collective is needed. Each core returns
per-batch-chunk partial sums of exp(32c-32); the host does the final
O(B) combine: sum across cores, margin-target correction (cos(theta+m)
needs only sqrt, no arccos), ln, weighted dot. No device collective.

Device per core: the whole fp8 weight shard stays resident in SBUF
(24.5KB/partition); DoubleRow matmuls (K=256 per instruction) fill
4-bank PSUM tiles [128b, 2048c]; ScalarE exp -> bf16; VectorE 2x-mode
running adds + one final reduce per batch chunk; single DMA out.
"""

import numpy as np
import ml_dtypes

import concourse.bass as bass
import concourse.tile as tile
from concourse import bacc, mybir
from concourse.bass_utils import run_bass_kernel_spmd

B = 512
D = 256
C = 100000
NCORES = 8
CSH = C // NCORES          # 12500 classes per core
# class tiles per core: six 2048-wide + one 256-wide = 12544
TILES = [(i * 2048, 2048) for i in range(6)] + [(12288, 256)]
CPAD = 12544
NPAD_TOT = (CPAD - CSH) * NCORES

M0 = 0.5
M_MIN = 0.25
SCALE = 32.0
SHIFT = 32.0               # fixed log-softmax shift (|logits| <= SCALE)
FP8_PRESCALE = 8.0         # both operands scaled by 8 -> matmul gives 64*cos

f32 = mybir.dt.float32
bf16 = mybir.dt.bfloat16
fp8 = mybir.dt.float8e4

NBC = B // 128             # 4 batch chunks

_cached_nc = None
_last_results = None


def _build():
    global _cached_nc
    if _cached_nc is not None:
        return _cached_nc

    nc = bacc.Bacc(
        "TRN2", target_bir_lowering=False, debug=False, num_devices=NCORES
    )

    # [p, j, c] with contraction index k = j*128 + p
    wnT_d = nc.dram_tensor("wnT", [128, 2, CPAD], fp8, kind="ExternalInput")
    featnT_d = nc.dram_tensor("featnT", [128, 2, B], fp8, kind="ExternalInput")
    out_d = nc.dram_tensor("out", [128, NBC], f32, kind="ExternalOutput")

    with tile.TileContext(nc) as tc:
        with (
            tc.tile_pool(name="persist", bufs=1) as persist,
            tc.tile_pool(name="epool", bufs=3) as epool,
            tc.tile_pool(name="psum", bufs=2, space="PSUM") as psum,
        ):
            fsb = persist.tile([128, 2, B], fp8)
            nc.sync.dma_start(out=fsb[:], in_=featnT_d[:])

            wsb = persist.tile([128, 2, CPAD], fp8)
            # chunked loads so compute can start early
            NW = 16
            step = CPAD // NW  # 784
            for ci in range(NW):
                nc.sync.dma_start(
                    out=wsb[:, :, ci * step:(ci + 1) * step],
                    in_=wnT_d[:, :, ci * step:(ci + 1) * step],
                )

            bias_s = persist.tile([128, 1], f32)
            nc.gpsimd.memset(bias_s[:], -SHIFT)

            eacc = [
                persist.tile(
                    [128, 2048], bf16, tag=f"eacc{bc}", name=f"eacc{bc}"
                )
                for bc in range(NBC)
            ]

            S_all = persist.tile([128, NBC], f32)

            for bc in range(NBC):
                lhs = fsb[:, :, bc * 128:(bc + 1) * 128]
                for ti, (c0, cw) in enumerate(TILES):
                    ps = psum.tile([128, 2048], f32, tag="ps")
                    for j in range(0, cw, 512):
                        jw = min(512, cw - j)
                        nc.tensor.matmul(
                            ps[:, j:j + jw],
                            lhs,
                            wsb[:, :, c0 + j:c0 + j + jw],
                            start=True, stop=True,
                            perf_mode=mybir.MatmulPerfMode.DoubleRow,
                        )
                    esc = epool.tile([128, 2048], bf16, tag="esc")
                    nc.scalar.activation(
                        esc[:, :cw], ps[:, :cw],
                        mybir.ActivationFunctionType.Exp,
                        bias=bias_s[:], scale=SCALE / (FP8_PRESCALE**2),
                    )
                    if ti == 0:
                        # first tile initializes the accumulator (covers the
                        # full 2048 width; later tiles add elementwise)
                        nc.vector.tensor_copy(eacc[bc][:], esc[:])
                    else:
                        nc.vector.tensor_add(
                            eacc[bc][:, :cw], eacc[bc][:, :cw], esc[:, :cw]
                        )
                    if bc > 0 and ti == 1:
                        # previous chunk's column reduce, interleaved here so
                        # it doesn't stall the next sweep's accumulation
                        nc.vector.tensor_reduce(
                            S_all[:, bc - 1:bc],
                            eacc[bc - 1][:],
                            axis=mybir.AxisListType.X,
                            op=mybir.AluOpType.add,
                        )

            nc.vector.tensor_reduce(
                S_all[:, NBC - 1:NBC],
                eacc[NBC - 1][:],
                axis=mybir.AxisListType.X,
                op=mybir.AluOpType.add,
            )

            nc.sync.dma_start(out=out_d[:], in_=S_all[:])

    nc.compile()
    _cached_nc = nc
    return nc


def _host_prep(features, weight, weights, labels):
    """Everything O(B*D) / O(C*D) that is not the big matmul."""
    f = features.astype(np.float64)
    norms = np.sqrt((f * f).sum(axis=1))
    lo, hi = norms.min(), norms.max()
    denom = max(hi - lo, 1e-8)
    margins = np.clip(M_MIN + (M0 - M_MIN) * (norms - lo) / denom, M_MIN, M0)
    feat_n = f / np.maximum(norms, 1e-12)[:, None]

    wlab = weight[labels].astype(np.float64)
    wlab_n = wlab / np.maximum(
        np.sqrt((wlab * wlab).sum(axis=1)), 1e-12
    )[:, None]
    cos_t = np.clip((feat_n * wlab_n).sum(axis=1), -1.0 + 1e-7, 1.0 - 1e-7)
    cos_m = cos_t * np.cos(margins) - np.sqrt(1.0 - cos_t * cos_t) * np.sin(
        margins
    )
    t_logit = SCALE * cos_m
    corr = (
        np.exp(SCALE * cos_m - SHIFT)
        - np.exp(SCALE * cos_t - SHIFT)
        - NPAD_TOT * np.exp(-SHIFT)
    )
    coef = weights.astype(np.float64) / B
    return feat_n, corr, coef, t_logit


def _to_dr_layout(mat_t, width):
    """[D, X] f32 -> [128, 2, X] fp8 with k = j*128 + p."""
    a = mat_t.reshape(2, 128, width)          # [j, p, X]
    a = np.ascontiguousarray(a.transpose(1, 0, 2))  # [p, j, X]
    return a.astype(ml_dtypes.float8_e4m3)


def kernel(features, weight, weights, labels):
    global _last_results
    features = np.asarray(features, dtype=np.float32)
    weight = np.asarray(weight, dtype=np.float32)
    weights = np.asarray(weights, dtype=np.float32)
    labels = np.asarray(labels).astype(np.int64)

    feat_n, corr, coef, t_logit = _host_prep(features, weight, weights, labels)

    wn = weight / np.maximum(
        np.linalg.norm(weight, axis=1, keepdims=True), 1e-12
    )
    featnT = np.ascontiguousarray(feat_n.T.astype(np.float32)) * FP8_PRESCALE
    featnT8 = _to_dr_layout(featnT, B)

    in_maps = []
    for i in range(NCORES):
        sh = wn[i * CSH:(i + 1) * CSH]  # [CSH, D]
        wt = np.zeros((D, CPAD), dtype=np.float32)
        wt[:, :CSH] = sh.T * FP8_PRESCALE
        in_maps.append(
            {"wnT": _to_dr_layout(wt, CPAD), "featnT": featnT8}
        )

    nc = _build()
    res = run_bass_kernel_spmd(nc, in_maps, list(range(NCORES)))
    _last_results = res

    # ---- host combine ----
    S = np.zeros(B, dtype=np.float64)
    for i in range(NCORES):
        sc = np.asarray(res.results[i]["out"], dtype=np.float64)  # [128, 4]
        for bc in range(NBC):
            S[bc * 128:(bc + 1) * 128] += sc[:, bc]

    Z = S + corr
    per = SHIFT + np.log(Z) - t_logit
    loss = float((coef * per).sum())
    return np.array(loss, dtype=np.float32)
